# revision 2
# baseline (speedup 1.0000x reference)
"""GQA attention block (wq/wk/wv -> RoPE -> attention -> wo) on 8 TRN2 cores, v2.

Sharding: tensor-parallel over heads. Core j owns kv-head j and q-heads
{j, j+8, j+16, j+24} as two packs (j, j+8) and (j+16, j+24). Each core emits a
full [S, E] fp16 partial of the output projection; partials summed on host.

Key scheduling/PE ideas vs v1:
  - Scores run as row-tiled concurrent matmul pairs (two K=64 heads in PE row
    groups 0:64 / 64:128) and AV as col-tiled concurrent pairs (two M=64 heads
    in PE col groups, each with its own moving stream) -> ~2x attention matmul
    throughput on HW.
  - exp is one [128, 1024] activation per (pack, kt) covering both heads
    (scores tile spans two adjacent PSUM banks); denominator comes from
    fp16 running sums of et maintained on DVE (even kt) and Pool (odd kt),
    reduced by tiny col-paired ones-matmuls at unit end. No ones-column in V,
    so AV pairs use the full 128 PE columns.
  - RoPE pair-swap is an intra-32-partition stream_shuffle (head dims are
    host-permuted in 16-even/16-odd blocks), no SBUF-SBUF swap DMAs.
  - xq resident in SBUF; x/weight DMAs round-robin across the sync and scalar
    HWDGE rings ordered by first use (xkv before xq, wo last).
  - Output partial in fp16 (half the writeback bytes).
"""

import sys

sys.path.insert(0, "/opt/trn_rl_repo")

from contextlib import ExitStack

import ml_dtypes
import numpy as np

import concourse.bacc as bacc
import concourse.bass as bass
import concourse.tile as tile
from concourse import mybir
from concourse.bass_utils import run_bass_kernel_spmd

P = 128
S = 2048   # sequence length
E = 2048   # embed dim
D = 64     # head dim
EK = E // P    # 16 contraction tiles for projections
SK = S // P    # 16 key tiles for attention
NSLICE = 4
QW = S // NSLICE  # 512
NCORES = 8
F32 = mybir.dt.float32
BF16 = mybir.dt.bfloat16
FP16 = mybir.dt.float16
AF = mybir.ActivationFunctionType
BF16NP = ml_dtypes.bfloat16
FP16NP = np.float16

# intra-32 pair swap: i <-> (i+16) % 32 within each 32-partition quadrant
SWAP_MASK = [(i + 16) % 32 for i in range(32)]


def build_bass(repeat=1):
    nc = bacc.Bacc()
    xqT = nc.declare_dram_parameter("xqT", [E, S], BF16, isOutput=False)
    xkvT = nc.declare_dram_parameter("xkvT", [E, S], BF16, isOutput=False)
    wqT = nc.declare_dram_parameter("wqT", [E, 256], BF16, isOutput=False)
    wkvT = nc.declare_dram_parameter("wkvT", [E, P], BF16, isOutput=False)
    woT = nc.declare_dram_parameter("woT", [256, E], BF16, isOutput=False)
    rqc = nc.declare_dram_parameter("rqc", [D, S], BF16, isOutput=False)
    rqs = nc.declare_dram_parameter("rqs", [D, S], BF16, isOutput=False)
    rkc = nc.declare_dram_parameter("rkc", [D, S], BF16, isOutput=False)
    rks = nc.declare_dram_parameter("rks", [D, S], BF16, isOutput=False)
    mbias = nc.declare_dram_parameter("mbias", [P, SK], F32, isOutput=False)
    ident = nc.declare_dram_parameter("ident", [P, P], BF16, isOutput=False)
    outp = nc.declare_dram_parameter("out_partial", [S, E], FP16, isOutput=True)

    with ExitStack() as ctx:
        tc = ctx.enter_context(tile.TileContext(nc))
        persist = ctx.enter_context(tc.tile_pool(name="persist", bufs=1))

        wq_sb = persist.tile([P, EK, 256], BF16, tag="wq_sb")
        wkv_sb = persist.tile([P, EK, P], BF16, tag="wkv_sb")
        wo_sb = persist.tile([P, 2, S], BF16, tag="wo_sb")
        rq_c = persist.tile([P, S], BF16, tag="rq_c")
        rq_s = persist.tile([P, S], BF16, tag="rq_s")
        rk_c = persist.tile([D, S], BF16, tag="rk_c")
        rk_s = persist.tile([D, S], BF16, tag="rk_s")
        mb_sb = persist.tile([P, SK], F32, tag="mb_sb")
        id_sb = persist.tile([P, P], BF16, tag="id_sb")
        ones_c = persist.tile([P, 1], BF16, tag="ones_c")
        ones64 = persist.tile([P, D], FP16, tag="ones64")
        qt = [persist.tile([P, S], BF16, tag=f"qt{i}", name=f"qt{i}") for i in range(2)]
        ktdup = persist.tile([P, S], BF16, tag="ktdup")
        kv_sb = persist.tile([P, S], BF16, tag="kv_sb")
        v_sb = persist.tile([P, SK, D], BF16, tag="v_sb")
        oP = [persist.tile([P, S], BF16, tag=f"oP{i}", name=f"oP{i}") for i in range(2)]

        nc.vector.memset(ones_c[:], 1.0)
        nc.vector.memset(ones64[:], 1.0)

        # ---- input DMAs, round-robin across the two HWDGE rings, in order of
        # first use: wkv/wq -> xkv -> rope tables/mask/ident -> xq -> wo
        nc.sync.dma_start(
            out=wkv_sb[:], in_=wkvT.ap().rearrange("(k p) c -> p k c", p=P)
        )
        wq_r = wqT.ap().rearrange("(k p) c -> p k c", p=P)
        nc.scalar.dma_start(out=wq_sb[:], in_=wq_r[:])
        nc.scalar.dma_start(out=mb_sb[:], in_=mbias[:])
        nc.scalar.dma_start(out=id_sb[:], in_=ident[:])

        nc.sync.dma_start(out=rk_c[:], in_=rkc[:])
        nc.scalar.dma_start(out=rk_s[:], in_=rks[:])
        nc.sync.dma_start(out=rq_c[0:D, :], in_=rqc[:])
        nc.scalar.dma_start(out=rq_s[0:D, :], in_=rqs[:])
        nc.vector.tensor_copy(rq_c[D:P, :], rq_c[0:D, :])
        nc.vector.tensor_copy(rq_s[D:P, :], rq_s[0:D, :])
        nc.scalar.dma_start(
            out=wo_sb[:], in_=woT.ap().rearrange("(k p) c -> p k c", p=P)
        )
        xkv_r = xkvT.ap().rearrange("(k p) s -> p k s", p=P)
        xq_r = xqT.ap().rearrange("(k p) s -> p k s", p=P)
        xkvp = ctx.enter_context(tc.tile_pool(name="xkvp", bufs=4))
        xqp = ctx.enter_context(tc.tile_pool(name="xqp", bufs=4))
        swp = ctx.enter_context(tc.tile_pool(name="swp", bufs=2))

        for rep in range(repeat):
            nm = f"r{rep}"
            # ============ Phase 1: projections (KV then Q), RoPE, V ============
            with tc.tile_pool(name="projp", bufs=4, space="PSUM") as projp:
                # KV projection, both halves concurrent; each xkv tile read once
                kvg = [
                    projp.tile([P, 2, QW], F32, tag="proj", name=f"{nm}_kv{h}")
                    for h in range(2)
                ]
                for k2 in range(EK // 2):
                    eng = nc.sync if k2 % 2 == 0 else nc.scalar
                    xt = xkvp.tile([P, 2, S], BF16, tag="xkv", name=f"{nm}_xkv{k2}")
                    eng.dma_start(out=xt[:], in_=xkv_r[:, k2 * 2:k2 * 2 + 2, :])
                    for b in range(2):
                        k = k2 * 2 + b
                        for h in range(2):
                            for c in range(2):
                                nc.tensor.matmul(
                                    kvg[h][:, c, :],
                                    wkv_sb[:, k, :],
                                    xt[:, b, h * 1024 + c * QW:
                                       h * 1024 + (c + 1) * QW],
                                    start=(k == 0),
                                    stop=(k == EK - 1),
                                )
                for h in range(2):
                    nc.vector.tensor_copy(
                        kv_sb[:, h * 1024:(h + 1) * 1024], kvg[h][:]
                    )

                # K rope: rows 0:64 of kv_sb -> ktdup[0:64], dup to [64:128]
                swk = swp.tile([P, S], BF16, tag="sw", name=f"{nm}_swk")
                nc.vector.stream_shuffle(swk[0:D, :], kv_sb[0:D, :], SWAP_MASK)
                nc.vector.tensor_mul(ktdup[0:D, :], kv_sb[0:D, :], rk_c[:])
                nc.vector.tensor_mul(swk[0:D, :], swk[0:D, :], rk_s[:])
                nc.vector.tensor_add(ktdup[0:D, :], ktdup[0:D, :], swk[0:D, :])
                nc.vector.tensor_copy(ktdup[D:P, :], ktdup[0:D, :])

                # V natural layout via PE transposes, batched into one psum
                # bank (bf16 view of an f32 proj-pool tile)
                tpt = projp.tile([P, 2, QW], F32, tag="proj", name=f"{nm}_tp")
                tp = tpt[:, 0, :].bitcast(BF16)  # [P, 1024] = [P, SK, D]
                for sk in range(SK):
                    nc.tensor.transpose(
                        tp[:, sk * D:(sk + 1) * D],
                        kv_sb[D:P, sk * P:(sk + 1) * P],
                        id_sb[D:P, D:P],
                    )
                nc.vector.tensor_copy(v_sb[:], tp[:])

                # Q projection: all 4 (pack, half) targets per k pass
                qg = [
                    [
                        projp.tile(
                            [P, 2, QW], F32, tag="proj", name=f"{nm}_q{p_}{h}"
                        )
                        for h in range(2)
                    ]
                    for p_ in range(2)
                ]
                for k2 in range(EK // 2):
                    eng = nc.scalar if k2 % 2 == 0 else nc.sync
                    xt = xqp.tile([P, 2, S], BF16, tag="xq", name=f"{nm}_xq{k2}")
                    eng.dma_start(out=xt[:], in_=xq_r[:, k2 * 2:k2 * 2 + 2, :])
                    for b in range(2):
                        k = k2 * 2 + b
                        for p_ in range(2):
                            for h in range(2):
                                for c in range(2):
                                    nc.tensor.matmul(
                                        qg[p_][h][:, c, :],
                                        wq_sb[:, k, p_ * P:(p_ + 1) * P],
                                        xt[:, b, h * 1024 + c * QW:
                                           h * 1024 + (c + 1) * QW],
                                        start=(k == 0),
                                        stop=(k == EK - 1),
                                    )
                for p_ in range(2):
                    for h in range(2):
                        nc.vector.tensor_copy(
                            qt[p_][:, h * 1024:(h + 1) * 1024], qg[p_][h][:]
                        )
                    # Q rope for this pack
                    sw = swp.tile([P, S], BF16, tag="sw", name=f"{nm}_sw{p_}")
                    nc.vector.stream_shuffle(sw[:], qt[p_][:], SWAP_MASK)
                    nc.vector.tensor_mul(qt[p_][:], qt[p_][:], rq_c[:])
                    nc.vector.tensor_mul(sw[:], sw[:], rq_s[:])
                    nc.vector.tensor_add(qt[p_][:], qt[p_][:], sw[:])

            # ============ Phase 2: attention + output projection ============
            with tc.tile_pool(name="scp", bufs=2, space="PSUM") as scp, \
                 tc.tile_pool(name="up", bufs=1, space="PSUM") as up, \
                 tc.tile_pool(name="auxp", bufs=1, space="PSUM") as auxp, \
                 tc.tile_pool(name="etp", bufs=4) as etp, \
                 tc.tile_pool(name="rcp", bufs=2) as rcp, \
                 tc.tile_pool(name="stgp", bufs=2) as stgp:

                def oproj_stile(st):
                    stg = stgp.tile([P, 4, QW], FP16, tag="stg", name=f"{nm}_stg{st}")
                    for cc in range(2):
                        wp = auxp.tile(
                            [P, 2, QW], F32, tag="aux", name=f"{nm}_wp{st}_{cc}"
                        )
                        for c2 in range(2):
                            c = cc * 2 + c2
                            nc.tensor.matmul(
                                wp[:, c2, :],
                                oP[0][:, st * P:(st + 1) * P],
                                wo_sb[:, 0, c * QW:(c + 1) * QW],
                                start=True,
                                stop=False,
                            )
                            nc.tensor.matmul(
                                wp[:, c2, :],
                                oP[1][:, st * P:(st + 1) * P],
                                wo_sb[:, 1, c * QW:(c + 1) * QW],
                                start=False,
                                stop=True,
                            )
                        nc.vector.tensor_copy(stg[:, cc * 2:cc * 2 + 2, :], wp[:])
                    nc.sync.dma_start(
                        out=outp[st * P:(st + 1) * P, :], in_=stg[:]
                    )

                for sl in range(NSLICE):
                    qw = slice(sl * QW, (sl + 1) * QW)
                    for pk in range(2):
                        un = f"{nm}_u{sl}{pk}"
                        U = up.tile([P, QW], F32, tag="u", name=un)
                        dn = up.tile([P, QW], F32, tag="den", name=un + "d")
                        for kt in range(SK):
                            ktw = slice(kt * P, (kt + 1) * P)
                            sc = scp.tile(
                                [P, 2, QW], F32, tag="sc", name=f"{un}_sc{kt}"
                            )
                            nc.tensor.matmul(
                                sc[:, 0, :], ktdup[0:D, ktw], qt[pk][0:D, qw],
                                start=True, stop=True, tile_position=(0, 0),
                            )
                            nc.tensor.matmul(
                                sc[:, 1, :], ktdup[D:P, ktw], qt[pk][D:P, qw],
                                start=True, stop=True, tile_position=(64, 0),
                            )
                            et = etp.tile(
                                [P, 2, QW], BF16, tag="et", name=f"{un}_et{kt}"
                            )
                            nc.scalar.activation(
                                et[:], sc[:], AF.Exp,
                                bias=mb_sb[:, kt:kt + 1], scale=0.125,
                            )
                            nc.tensor.matmul(
                                U[0:D, :], v_sb[:, kt, :], et[:, 0, :],
                                start=(kt == 0), stop=(kt == SK - 1),
                                tile_position=(0, 0),
                            )
                            nc.tensor.matmul(
                                U[D:P, :], v_sb[:, kt, :], et[:, 1, :],
                                start=(kt == 0), stop=(kt == SK - 1),
                                tile_position=(0, 64),
                            )
                            # denominator: col-paired ones-matmuls riding
                            # the same et streams, accumulated in one bank
                            nc.tensor.matmul(
                                dn[0:1, :], ones_c[:], et[:, 0, :],
                                start=(kt == 0), stop=(kt == SK - 1),
                                tile_position=(0, 0),
                            )
                            nc.tensor.matmul(
                                dn[64:65, :], ones_c[:], et[:, 1, :],
                                start=(kt == 0), stop=(kt == SK - 1),
                                tile_position=(0, 64),
                            )

                        # recip -> K=1 col-paired broadcast -> normalize
                        rc = rcp.tile([P, QW], FP16, tag="rc", name=un + "rc")
                        with nc.allow_low_precision(
                            reason="softmax denom recip at fp16"
                        ):
                            nc.vector.reciprocal(rc[0:1, :], dn[0:1, :])
                            nc.vector.reciprocal(rc[64:65, :], dn[64:65, :])
                        bt = auxp.tile([P, 2, QW], F32, tag="aux", name=un + "bt")
                        nc.tensor.matmul(
                            bt[0:D, 0, :], ones64[0:1, :], rc[0:1, :],
                            start=True, stop=True, tile_position=(0, 0),
                        )
                        nc.tensor.matmul(
                            bt[D:P, 0, :], ones64[64:65, :], rc[64:65, :],
                            start=True, stop=True, tile_position=(64, 64),
                        )
                        # DVE reads at most one PSUM operand: stage U in SBUF
                        u_sb = rcp.tile([P, QW], F32, tag="u_sb", name=un + "us")
                        nc.vector.tensor_copy(u_sb[:], U[:])
                        nc.vector.tensor_mul(oP[pk][:, qw], u_sb[:], bt[:, 0, :])

                    # output projection for this slice's s-tiles
                    for st in range(sl * 4, sl * 4 + 4):
                        oproj_stile(st)

    nc.compile()
    return nc


# rope pairing: within each 64-dim head, dims reordered as
# [evens 0:16 | odds 0:16 | evens 16:32 | odds 16:32] so the rope partner is
# always 16 partitions away inside a 32-partition block.
_PERM = np.concatenate([
    np.arange(0, 32, 2), np.arange(1, 32, 2),
    np.arange(32, 64, 2), np.arange(33, 64, 2),
])
# dest row -> rope pair index, and sin sign
_PAIR = np.concatenate([
    np.arange(16), np.arange(16), np.arange(16, 32), np.arange(16, 32)
])
_SIGN = np.concatenate([
    -np.ones(16), np.ones(16), -np.ones(16), np.ones(16)
])


def _rope_tables(cos, sin):
    # cos/sin: [S, 32] -> [64, S] dest-row tables
    c = np.ascontiguousarray(cos.T[_PAIR, :])
    s = np.ascontiguousarray(sin.T[_PAIR, :] * _SIGN[:, None])
    return c.astype(BF16NP), s.astype(BF16NP)


def _host_inputs(inputs):
    q = np.asarray(inputs["query_states"], np.float32)[0].T.astype(BF16NP)
    kv = np.asarray(inputs["key_value_states"], np.float32)[0].T.astype(BF16NP)
    wq = np.asarray(inputs["wq"], np.float32)
    wk = np.asarray(inputs["wk"], np.float32)
    wv = np.asarray(inputs["wv"], np.float32)
    wo = np.asarray(inputs["wo"], np.float32)
    mask = np.asarray(inputs["attention_mask"]).reshape(S)

    rq_c, rq_s = _rope_tables(
        np.asarray(inputs["cos_q"], np.float32), np.asarray(inputs["sin_q"], np.float32)
    )
    rk_c, rk_s = _rope_tables(
        np.asarray(inputs["cos_k"], np.float32), np.asarray(inputs["sin_k"], np.float32)
    )
    mb = np.where(mask, 0.0, -30000.0).astype(np.float32)
    mb = np.ascontiguousarray(mb.reshape(SK, P).T)  # [P, SK]
    ident = np.eye(P, dtype=BF16NP)

    shared = {
        "xqT": np.ascontiguousarray(q),
        "xkvT": np.ascontiguousarray(kv),
        "rqc": rq_c, "rqs": rq_s, "rkc": rk_c, "rks": rk_s,
        "mbias": mb, "ident": ident,
    }

    in_maps = []
    for j in range(NCORES):
        heads = [j, j + 8, j + 16, j + 24]
        wqTh = np.empty((E, 256), np.float32)
        for i, h in enumerate(heads):
            wqTh[:, i * D:(i + 1) * D] = wq[h * D + _PERM, :].T
        wk_p = wk[j * D + _PERM, :].T       # [E, 64]
        wv_p = wv[j * D:(j + 1) * D, :].T   # [E, 64] natural
        wkvTh = np.concatenate([wk_p, wv_p], axis=1)
        woTh = np.empty((256, E), np.float32)
        for slot, h in enumerate(heads):
            woTh[slot * D:(slot + 1) * D, :] = wo[:, h * D:(h + 1) * D].T
        in_maps.append({
            **shared,
            "wqT": np.ascontiguousarray(wqTh.astype(BF16NP)),
            "wkvT": np.ascontiguousarray(wkvTh.astype(BF16NP)),
            "woT": np.ascontiguousarray(woTh.astype(BF16NP)),
        })
    return in_maps


_NC_CACHE = {}


def _get_nc():
    if "nc" not in _NC_CACHE:
        _NC_CACHE["nc"] = build_bass()
    return _NC_CACHE["nc"]


def kernel(_trace=False, **inputs):
    nc = _get_nc()
    in_maps = _host_inputs(inputs)
    res = run_bass_kernel_spmd(
        nc, in_maps, core_ids=list(range(NCORES)), trace=_trace
    )
    out = np.zeros((S, E), np.float32)
    for r in res.results:
        out += r["out_partial"].astype(np.float32)
    if _trace:
        kernel.last_exec_time_ns = res.exec_time_ns
        kernel.last_results = res
    return out.reshape(1, S, E)


# revision 3
# speedup vs baseline: 8.6228x; 8.6228x over previous
"""GQA attention block (wq/wk/wv -> RoPE -> attention -> wo) on 8 TRN2 cores, v2.

Sharding: tensor-parallel over heads. Core j owns kv-head j and q-heads
{j, j+8, j+16, j+24} as two packs (j, j+8) and (j+16, j+24). Each core emits a
full [S, E] fp16 partial of the output projection; partials summed on host.

Key scheduling/PE ideas vs v1:
  - Scores run as row-tiled concurrent matmul pairs (two K=64 heads in PE row
    groups 0:64 / 64:128) and AV as col-tiled concurrent pairs (two M=64 heads
    in PE col groups, each with its own moving stream) -> ~2x attention matmul
    throughput on HW.
  - exp is one [128, 1024] activation per (pack, kt) covering both heads
    (scores tile spans two adjacent PSUM banks); denominator comes from
    fp16 running sums of et maintained on DVE (even kt) and Pool (odd kt),
    reduced by tiny col-paired ones-matmuls at unit end. No ones-column in V,
    so AV pairs use the full 128 PE columns.
  - RoPE pair-swap is an intra-32-partition stream_shuffle (head dims are
    host-permuted in 16-even/16-odd blocks), no SBUF-SBUF swap DMAs.
  - xq resident in SBUF; x/weight DMAs round-robin across the sync and scalar
    HWDGE rings ordered by first use (xkv before xq, wo last).
  - Output partial in fp16 (half the writeback bytes).
"""

import sys

sys.path.insert(0, "/opt/trn_rl_repo")

from contextlib import ExitStack

import ml_dtypes
import numpy as np

import concourse.bacc as bacc
import concourse.bass as bass
import concourse.tile as tile
from concourse import mybir
from concourse.bass_utils import run_bass_kernel_spmd

P = 128
S = 2048   # sequence length
E = 2048   # embed dim
D = 64     # head dim
EK = E // P    # 16 contraction tiles for projections
SK = S // P    # 16 key tiles for attention
NSLICE = 4
QW = S // NSLICE  # 512
NCORES = 8
F32 = mybir.dt.float32
BF16 = mybir.dt.bfloat16
FP16 = mybir.dt.float16
AF = mybir.ActivationFunctionType
BF16NP = ml_dtypes.bfloat16
FP16NP = np.float16

# intra-32 pair swap: i <-> (i+16) % 32 within each 32-partition quadrant
SWAP_MASK = [(i + 16) % 32 for i in range(32)]


def build_bass(repeat=1):
    nc = bacc.Bacc()
    xqT = nc.declare_dram_parameter("xqT", [E, S], BF16, isOutput=False)
    xkvT = nc.declare_dram_parameter("xkvT", [E, S], BF16, isOutput=False)
    wqT = nc.declare_dram_parameter("wqT", [E, 256], BF16, isOutput=False)
    wkvT = nc.declare_dram_parameter("wkvT", [E, P], BF16, isOutput=False)
    woT = nc.declare_dram_parameter("woT", [256, E], BF16, isOutput=False)
    rqc = nc.declare_dram_parameter("rqc", [D, S], BF16, isOutput=False)
    rqs = nc.declare_dram_parameter("rqs", [D, S], BF16, isOutput=False)
    rkc = nc.declare_dram_parameter("rkc", [D, S], BF16, isOutput=False)
    rks = nc.declare_dram_parameter("rks", [D, S], BF16, isOutput=False)
    mbias = nc.declare_dram_parameter("mbias", [P, SK], F32, isOutput=False)
    ident = nc.declare_dram_parameter("ident", [P, P], BF16, isOutput=False)
    outp = nc.declare_dram_parameter("out_partial", [S, E], FP16, isOutput=True)

    with ExitStack() as ctx:
        tc = ctx.enter_context(tile.TileContext(nc))
        persist = ctx.enter_context(tc.tile_pool(name="persist", bufs=1))

        wq_sb = persist.tile([P, EK, 256], BF16, tag="wq_sb")
        wkv_sb = persist.tile([P, EK, P], BF16, tag="wkv_sb")
        wo_sb = persist.tile([P, 2, S], BF16, tag="wo_sb")
        rq_c = persist.tile([P, S], BF16, tag="rq_c")
        rq_s = persist.tile([P, S], BF16, tag="rq_s")
        rk_c = persist.tile([D, S], BF16, tag="rk_c")
        rk_s = persist.tile([D, S], BF16, tag="rk_s")
        mb_sb = persist.tile([P, SK], F32, tag="mb_sb")
        id_sb = persist.tile([P, P], BF16, tag="id_sb")
        ones_c = persist.tile([P, 1], BF16, tag="ones_c")
        ones64 = persist.tile([P, D], FP16, tag="ones64")
        qt = [persist.tile([P, S], BF16, tag=f"qt{i}", name=f"qt{i}") for i in range(2)]
        ktdup = persist.tile([P, S], BF16, tag="ktdup")
        kv_sb = persist.tile([P, S], BF16, tag="kv_sb")
        v_sb = persist.tile([P, SK, D], BF16, tag="v_sb")
        oP = [persist.tile([P, S], BF16, tag=f"oP{i}", name=f"oP{i}") for i in range(2)]

        nc.vector.memset(ones_c[:], 1.0)
        nc.vector.memset(ones64[:], 1.0)

        # ---- input DMAs, round-robin across the two HWDGE rings, in order of
        # first use: wkv/wq -> xkv -> rope tables/mask/ident -> xq -> wo
        nc.sync.dma_start(
            out=wkv_sb[:], in_=wkvT.ap().rearrange("(k p) c -> p k c", p=P)
        )
        wq_r = wqT.ap().rearrange("(k p) c -> p k c", p=P)
        nc.scalar.dma_start(out=wq_sb[:], in_=wq_r[:])
        nc.scalar.dma_start(out=mb_sb[:], in_=mbias[:])
        nc.scalar.dma_start(out=id_sb[:], in_=ident[:])

        nc.sync.dma_start(out=rk_c[:], in_=rkc[:])
        nc.scalar.dma_start(out=rk_s[:], in_=rks[:])
        nc.sync.dma_start(out=rq_c[0:D, :], in_=rqc[:])
        nc.scalar.dma_start(out=rq_s[0:D, :], in_=rqs[:])
        nc.vector.tensor_copy(rq_c[D:P, :], rq_c[0:D, :])
        nc.vector.tensor_copy(rq_s[D:P, :], rq_s[0:D, :])
        nc.scalar.dma_start(
            out=wo_sb[:], in_=woT.ap().rearrange("(k p) c -> p k c", p=P)
        )
        xkv_r = xkvT.ap().rearrange("(k p) s -> p k s", p=P)
        xq_r = xqT.ap().rearrange("(k p) s -> p k s", p=P)
        xkvp = ctx.enter_context(tc.tile_pool(name="xkvp", bufs=4))
        xqp = ctx.enter_context(tc.tile_pool(name="xqp", bufs=4))
        swp = ctx.enter_context(tc.tile_pool(name="swp", bufs=2))

        for rep in range(repeat):
            nm = f"r{rep}"
            # ============ Phase 1: projections (KV then Q), RoPE, V ============
            with tc.tile_pool(name="projp", bufs=4, space="PSUM") as projp:
                # KV projection, both halves concurrent; each xkv tile read once
                kvg = [
                    projp.tile([P, 2, QW], F32, tag="proj", name=f"{nm}_kv{h}")
                    for h in range(2)
                ]
                for k2 in range(EK // 2):
                    eng = nc.sync if k2 % 2 == 0 else nc.scalar
                    xt = xkvp.tile([P, 2, S], BF16, tag="xkv", name=f"{nm}_xkv{k2}")
                    eng.dma_start(out=xt[:], in_=xkv_r[:, k2 * 2:k2 * 2 + 2, :])
                    for b in range(2):
                        k = k2 * 2 + b
                        for h in range(2):
                            for c in range(2):
                                nc.tensor.matmul(
                                    kvg[h][:, c, :],
                                    wkv_sb[:, k, :],
                                    xt[:, b, h * 1024 + c * QW:
                                       h * 1024 + (c + 1) * QW],
                                    start=(k == 0),
                                    stop=(k == EK - 1),
                                )
                for h in range(2):
                    nc.vector.tensor_copy(
                        kv_sb[:, h * 1024:(h + 1) * 1024], kvg[h][:]
                    )

                # K rope: rows 0:64 of kv_sb -> ktdup[0:64], dup to [64:128]
                swk = swp.tile([P, S], BF16, tag="sw", name=f"{nm}_swk")
                nc.vector.stream_shuffle(swk[0:D, :], kv_sb[0:D, :], SWAP_MASK)
                nc.vector.tensor_mul(ktdup[0:D, :], kv_sb[0:D, :], rk_c[:])
                nc.vector.tensor_mul(swk[0:D, :], swk[0:D, :], rk_s[:])
                nc.vector.tensor_add(ktdup[0:D, :], ktdup[0:D, :], swk[0:D, :])
                nc.vector.tensor_copy(ktdup[D:P, :], ktdup[0:D, :])

                # V natural layout via PE transposes, batched into one psum
                # bank (bf16 view of an f32 proj-pool tile)
                tpt = projp.tile([P, 2, QW], F32, tag="proj", name=f"{nm}_tp")
                tp = tpt[:, 0, :].bitcast(BF16)  # [P, 1024] = [P, SK, D]
                for sk in range(SK):
                    nc.tensor.transpose(
                        tp[:, sk * D:(sk + 1) * D],
                        kv_sb[D:P, sk * P:(sk + 1) * P],
                        id_sb[D:P, D:P],
                    )
                nc.vector.tensor_copy(v_sb[:], tp[:])

                # Q projection: all 4 (pack, half) targets per k pass
                qg = [
                    [
                        projp.tile(
                            [P, 2, QW], F32, tag="proj", name=f"{nm}_q{p_}{h}"
                        )
                        for h in range(2)
                    ]
                    for p_ in range(2)
                ]
                for k2 in range(EK // 2):
                    eng = nc.scalar if k2 % 2 == 0 else nc.sync
                    xt = xqp.tile([P, 2, S], BF16, tag="xq", name=f"{nm}_xq{k2}")
                    eng.dma_start(out=xt[:], in_=xq_r[:, k2 * 2:k2 * 2 + 2, :])
                    for b in range(2):
                        k = k2 * 2 + b
                        for p_ in range(2):
                            for h in range(2):
                                for c in range(2):
                                    nc.tensor.matmul(
                                        qg[p_][h][:, c, :],
                                        wq_sb[:, k, p_ * P:(p_ + 1) * P],
                                        xt[:, b, h * 1024 + c * QW:
                                           h * 1024 + (c + 1) * QW],
                                        start=(k == 0),
                                        stop=(k == EK - 1),
                                    )
                for p_ in range(2):
                    for h in range(2):
                        nc.vector.tensor_copy(
                            qt[p_][:, h * 1024:(h + 1) * 1024], qg[p_][h][:]
                        )
                    # Q rope for this pack
                    sw = swp.tile([P, S], BF16, tag="sw", name=f"{nm}_sw{p_}")
                    nc.vector.stream_shuffle(sw[:], qt[p_][:], SWAP_MASK)
                    nc.vector.tensor_mul(qt[p_][:], qt[p_][:], rq_c[:])
                    nc.vector.tensor_mul(sw[:], sw[:], rq_s[:])
                    nc.vector.tensor_add(qt[p_][:], qt[p_][:], sw[:])

            # ============ Phase 2: attention + output projection ============
            with tc.tile_pool(name="scp", bufs=2, space="PSUM") as scp, \
                 tc.tile_pool(name="up", bufs=1, space="PSUM") as up, \
                 tc.tile_pool(name="auxp", bufs=1, space="PSUM") as auxp, \
                 tc.tile_pool(name="etp", bufs=4) as etp, \
                 tc.tile_pool(name="rcp", bufs=2) as rcp, \
                 tc.tile_pool(name="stgp", bufs=2) as stgp:

                def oproj_stile(st):
                    stg = stgp.tile([P, 4, QW], FP16, tag="stg", name=f"{nm}_stg{st}")
                    for cc in range(2):
                        wp = auxp.tile(
                            [P, 2, QW], F32, tag="aux", name=f"{nm}_wp{st}_{cc}"
                        )
                        for c2 in range(2):
                            c = cc * 2 + c2
                            nc.tensor.matmul(
                                wp[:, c2, :],
                                oP[0][:, st * P:(st + 1) * P],
                                wo_sb[:, 0, c * QW:(c + 1) * QW],
                                start=True,
                                stop=False,
                            )
                            nc.tensor.matmul(
                                wp[:, c2, :],
                                oP[1][:, st * P:(st + 1) * P],
                                wo_sb[:, 1, c * QW:(c + 1) * QW],
                                start=False,
                                stop=True,
                            )
                        nc.vector.tensor_copy(stg[:, cc * 2:cc * 2 + 2, :], wp[:])
                    nc.sync.dma_start(
                        out=outp[st * P:(st + 1) * P, :], in_=stg[:]
                    )

                for sl in range(NSLICE):
                    qw = slice(sl * QW, (sl + 1) * QW)
                    for pk in range(2):
                        un = f"{nm}_u{sl}{pk}"
                        U = up.tile([P, QW], F32, tag="u", name=un)
                        dn = up.tile([P, QW], F32, tag="den", name=un + "d")
                        # scores(kt)+exp(kt) are emitted BEFORE AV/den of
                        # kt-1 so a scores pair outranks the AV/den matmuls
                        # that become ready mid-pair - keeps both members
                        # back-to-back in the PE queue (they run concurrently
                        # in disjoint PE row groups on HW).
                        ets = []

                        def av_den(kt):
                            et = ets[kt]
                            nc.tensor.matmul(
                                U[0:D, :], v_sb[:, kt, :], et[:, 0, :],
                                start=(kt == 0), stop=(kt == SK - 1),
                                tile_position=(0, 0),
                            )
                            nc.tensor.matmul(
                                U[D:P, :], v_sb[:, kt, :], et[:, 1, :],
                                start=(kt == 0), stop=(kt == SK - 1),
                                tile_position=(0, 64),
                            )
                            nc.tensor.matmul(
                                dn[0:1, :], ones_c[:], et[:, 0, :],
                                start=(kt == 0), stop=(kt == SK - 1),
                                tile_position=(0, 0),
                            )
                            nc.tensor.matmul(
                                dn[64:65, :], ones_c[:], et[:, 1, :],
                                start=(kt == 0), stop=(kt == SK - 1),
                                tile_position=(0, 64),
                            )

                        for kt in range(SK):
                            ktw = slice(kt * P, (kt + 1) * P)
                            sc = scp.tile(
                                [P, 2, QW], F32, tag="sc", name=f"{un}_sc{kt}"
                            )
                            nc.tensor.matmul(
                                sc[:, 0, :], ktdup[0:D, ktw], qt[pk][0:D, qw],
                                start=True, stop=True, tile_position=(0, 0),
                            )
                            nc.tensor.matmul(
                                sc[:, 1, :], ktdup[D:P, ktw], qt[pk][D:P, qw],
                                start=True, stop=True, tile_position=(64, 0),
                            )
                            et = etp.tile(
                                [P, 2, QW], BF16, tag="et", name=f"{un}_et{kt}"
                            )
                            nc.scalar.activation(
                                et[:], sc[:], AF.Exp,
                                bias=mb_sb[:, kt:kt + 1], scale=0.125,
                            )
                            ets.append(et)
                            if kt > 0:
                                av_den(kt - 1)
                        av_den(SK - 1)

                        # recip -> K=1 col-paired broadcast -> normalize
                        rc = rcp.tile([P, QW], FP16, tag="rc", name=un + "rc")
                        with nc.allow_low_precision(
                            reason="softmax denom recip at fp16"
                        ):
                            nc.vector.reciprocal(rc[0:1, :], dn[0:1, :])
                            nc.vector.reciprocal(rc[64:65, :], dn[64:65, :])
                        bt = auxp.tile([P, 2, QW], F32, tag="aux", name=un + "bt")
                        nc.tensor.matmul(
                            bt[0:D, 0, :], ones64[0:1, :], rc[0:1, :],
                            start=True, stop=True, tile_position=(0, 0),
                        )
                        nc.tensor.matmul(
                            bt[D:P, 0, :], ones64[64:65, :], rc[64:65, :],
                            start=True, stop=True, tile_position=(64, 64),
                        )
                        # DVE reads at most one PSUM operand: stage U in SBUF
                        u_sb = rcp.tile([P, QW], F32, tag="u_sb", name=un + "us")
                        nc.vector.tensor_copy(u_sb[:], U[:])
                        nc.vector.tensor_mul(oP[pk][:, qw], u_sb[:], bt[:, 0, :])

                    # output projection for this slice's s-tiles
                    for st in range(sl * 4, sl * 4 + 4):
                        oproj_stile(st)

    nc.compile()
    return nc


# rope pairing: within each 64-dim head, dims reordered as
# [evens 0:16 | odds 0:16 | evens 16:32 | odds 16:32] so the rope partner is
# always 16 partitions away inside a 32-partition block.
_PERM = np.concatenate([
    np.arange(0, 32, 2), np.arange(1, 32, 2),
    np.arange(32, 64, 2), np.arange(33, 64, 2),
])
# dest row -> rope pair index, and sin sign
_PAIR = np.concatenate([
    np.arange(16), np.arange(16), np.arange(16, 32), np.arange(16, 32)
])
_SIGN = np.concatenate([
    -np.ones(16), np.ones(16), -np.ones(16), np.ones(16)
])


def _rope_tables(cos, sin):
    # cos/sin: [S, 32] -> [64, S] dest-row tables
    c = np.ascontiguousarray(cos.T[_PAIR, :])
    s = np.ascontiguousarray(sin.T[_PAIR, :] * _SIGN[:, None])
    return c.astype(BF16NP), s.astype(BF16NP)


def _host_inputs(inputs):
    q = np.asarray(inputs["query_states"], np.float32)[0].T.astype(BF16NP)
    kv = np.asarray(inputs["key_value_states"], np.float32)[0].T.astype(BF16NP)
    wq = np.asarray(inputs["wq"], np.float32)
    wk = np.asarray(inputs["wk"], np.float32)
    wv = np.asarray(inputs["wv"], np.float32)
    wo = np.asarray(inputs["wo"], np.float32)
    mask = np.asarray(inputs["attention_mask"]).reshape(S)

    rq_c, rq_s = _rope_tables(
        np.asarray(inputs["cos_q"], np.float32), np.asarray(inputs["sin_q"], np.float32)
    )
    rk_c, rk_s = _rope_tables(
        np.asarray(inputs["cos_k"], np.float32), np.asarray(inputs["sin_k"], np.float32)
    )
    mb = np.where(mask, 0.0, -30000.0).astype(np.float32)
    mb = np.ascontiguousarray(mb.reshape(SK, P).T)  # [P, SK]
    ident = np.eye(P, dtype=BF16NP)

    shared = {
        "xqT": np.ascontiguousarray(q),
        "xkvT": np.ascontiguousarray(kv),
        "rqc": rq_c, "rqs": rq_s, "rkc": rk_c, "rks": rk_s,
        "mbias": mb, "ident": ident,
    }

    in_maps = []
    for j in range(NCORES):
        heads = [j, j + 8, j + 16, j + 24]
        wqTh = np.empty((E, 256), np.float32)
        for i, h in enumerate(heads):
            wqTh[:, i * D:(i + 1) * D] = wq[h * D + _PERM, :].T
        wk_p = wk[j * D + _PERM, :].T       # [E, 64]
        wv_p = wv[j * D:(j + 1) * D, :].T   # [E, 64] natural
        wkvTh = np.concatenate([wk_p, wv_p], axis=1)
        woTh = np.empty((256, E), np.float32)
        for slot, h in enumerate(heads):
            woTh[slot * D:(slot + 1) * D, :] = wo[:, h * D:(h + 1) * D].T
        in_maps.append({
            **shared,
            "wqT": np.ascontiguousarray(wqTh.astype(BF16NP)),
            "wkvT": np.ascontiguousarray(wkvTh.astype(BF16NP)),
            "woT": np.ascontiguousarray(woTh.astype(BF16NP)),
        })
    return in_maps


_NC_CACHE = {}


def _get_nc():
    if "nc" not in _NC_CACHE:
        _NC_CACHE["nc"] = build_bass()
    return _NC_CACHE["nc"]


def kernel(_trace=False, **inputs):
    nc = _get_nc()
    in_maps = _host_inputs(inputs)
    res = run_bass_kernel_spmd(
        nc, in_maps, core_ids=list(range(NCORES)), trace=_trace
    )
    out = np.zeros((S, E), np.float32)
    for r in res.results:
        out += r["out_partial"].astype(np.float32)
    if _trace:
        kernel.last_exec_time_ns = res.exec_time_ns
        kernel.last_results = res
    return out.reshape(1, S, E)
